# revision 13
# baseline (speedup 1.0000x reference)
"""BiDAF attention-flow kernel for 8 Trainium2 NeuronCores (Bass/Tile).

Data-parallel over batch: B=32 -> 4 batches per core on 8 cores.

Math (per batch b):
  sim[i,j] = s_proj[i] + t_proj[j] + sum_d S[i,d]*wm[d]*T[j,d]
  A        = softmax_j(sim)            (row-constant s_proj cancels)
  source_target = A @ T
  w[i]     = exp(max_j sim[i,j]) ; tgt_attn = w / sum(w)
  target_source = tgt_attn @ S         (one row, broadcast over rows)
  out      = [S | source_target | S*source_target | S*target_source]

Device strategy:
  - Host pre-transposes S,T to d-major layout (pure layout marshalling) so the
    similarity matmuls (contraction over d) need no on-device transposes.
  - sim^T (j on partitions) computed on PE -> exp on ACT with per-partition
    bias = t_proj[j]  -> E^T used as lhsT of the A@T matmul.
  - sim (i on partitions) computed on PE (plus rank-1 t_proj broadcast via a
    k=1 matmul) only to get the row max on DVE.
  - softmax normalization deferred: a ones-column appended to T gives the row
    sums in column 400 of the A@T matmul output; scaled on ACT via
    activation(Copy, scale=1/rowsum).
  - All big matmuls run as float32r (full PE rate for fp32 data).
"""

import sys
import types

import numpy as np

# concourse is importable via the axon sitecustomize path; fall back to /opt.
try:
    import concourse.bass as bass
except ImportError:  # pragma: no cover
    sys.path.insert(0, "/opt/trn_rl_repo")
    import concourse.bass as bass

import concourse.mybir as mybir
import concourse.tile as tile
from concourse.bass import ts
from concourse.bass_utils import run_bass_kernel_spmd

B, LS, LT, D = 32, 512, 512, 400
N_CORES = 8
BL = B // N_CORES  # batches per core
F32 = mybir.dt.float32
F32R = mybir.dt.float32r
EXP = mybir.ActivationFunctionType.Exp
AX = mybir.AxisListType.X


def _split_multi_waits(nc: bass.Bass) -> None:
    """This walrus build encodes at most ONE sync-wait per instruction
    ("Too many sync wait commands" in setupSyncWait).  Tile's wait pass can
    attach several sem-waits to one instruction; hoist the extras onto
    same-engine NoOp carriers immediately before it (the NX sequencer
    executes the waits in order, so semantics are identical)."""
    ctr = 0
    for fn in nc.m.functions:
        for bb in fn.blocks:
            if not any(
                i.sync_info is not None and len(i.sync_info.on_wait) > 1
                for i in bb.instructions
            ):
                continue
            new_insts = []
            for inst in bb.instructions:
                si = inst.sync_info
                if si is not None and len(si.on_wait) > 1:
                    waits = list(si.on_wait)
                    for w in waits[:-1]:
                        ctr += 1
                        nop = mybir.InstNoOp(
                            name=f"splitw-{ctr}",
                            engine=inst.engine,
                            sync_info=mybir.SyncInfo(on_wait=[w], on_update=[]),
                            bass_nofuse=True,
                        )
                        nc.register_instruction(nop, overwrite=True)
                        new_insts.append(nop)
                    del si.on_wait[:-1]
                new_insts.append(inst)
            bb.instructions[:] = new_insts


def _r(ap):
    """View an fp32 AP as float32r so the matmul runs at full PE rate."""
    if ap.dtype == F32R:
        return ap
    return ap.bitcast(F32R)


def build_program() -> bass.Bass:
    nc = bass.Bass("TRN2", target_bir_lowering=False, debug=False)

    # Per-core DRAM I/O (host feeds per-core shards).
    srow_h = nc.dram_tensor("srow", [BL, LS, 404], F32R, kind="ExternalInput").ap()
    trow_h = nc.dram_tensor("trow", [BL, LT, 404], F32R, kind="ExternalInput").ap()
    st_h = nc.dram_tensor("st", [BL, D + 2, LS], F32R, kind="ExternalInput").ap()
    tt_h = nc.dram_tensor("tt", [BL, D + 2, LT], F32R, kind="ExternalInput").ap()
    wcols_h = nc.dram_tensor("wcols", [128, 12], F32R, kind="ExternalInput").ap()
    ones_h = nc.dram_tensor("onesr", [1, 512], F32R, kind="ExternalInput").ap()
    out_h = nc.dram_tensor("out", [BL, LS, 1600], F32, kind="ExternalOutput").ap()

    KC = [128, 128, 128, 18]  # d chunks + 2 affine rows (402 = 3*128 + 18)

    with tile.TileContext(nc) as tc:
        with (
            tc.tile_pool(name="singles", bufs=1) as singles,
            tc.tile_pool(name="pin", bufs=3) as pin,
            tc.tile_pool(name="pet", bufs=2) as pet,
            tc.tile_pool(name="pout", bufs=2) as pout,
            tc.tile_pool(name="psml", bufs=3) as psml,
            tc.tile_pool(name="ptiny", bufs=6) as ptiny,
            tc.tile_pool(name="pbig_ps", bufs=4, space="PSUM") as pbig_ps,
            tc.tile_pool(name="psml_ps", bufs=3, space="PSUM") as psml_ps,
        ):
            ones_row = singles.tile([1, 512], F32R)
            nc.sync.dma_start(out=ones_row[:], in_=ones_h)
            wcols = singles.tile([128, 12], F32R)
            nc.sync.dma_start(out=wcols[:], in_=wcols_h)

            state = {}

            def prologue(b):
                """Batch b inputs + projections; emitted one batch ahead so
                ACT/DVE/DMA prep overlaps the previous batch's PE main work."""
                srow = pin.tile([128, 4, 404], F32R, tag="srow")
                nc.sync.dma_start(
                    out=srow[:], in_=srow_h[b].rearrange("(i p) c -> p i c", p=128)
                )
                trow = pin.tile([128, 4, 404], F32R, tag="trow")
                nc.sync.dma_start(
                    out=trow[:], in_=trow_h[b].rearrange("(j p) c -> p j c", p=128)
                )
                stt = pin.tile([128, 4, 512], F32R, tag="stt")
                ttt = pin.tile([128, 4, 512], F32R, tag="ttt")
                if b == 0:
                    for kc in range(3):
                        nc.sync.dma_start(
                            out=ttt[:, kc, :], in_=tt_h[b, ts(kc, 128), :]
                        )
                    for kc in range(3):
                        nc.sync.dma_start(
                            out=stt[:, kc, :], in_=st_h[b, ts(kc, 128), :]
                        )
                else:
                    nc.sync.dma_start(
                        out=stt[:, 0:3, :],
                        in_=st_h[b, 0:384, :].rearrange("(k p) c -> p k c", p=128),
                    )
                    nc.sync.dma_start(
                        out=ttt[:, 0:3, :],
                        in_=tt_h[b, 0:384, :].rearrange("(k p) c -> p k c", p=128),
                    )
                nc.sync.dma_start(out=stt[0:18, 3, :], in_=st_h[b, 384:402, :])
                nc.sync.dma_start(out=ttt[0:18, 3, :], in_=tt_h[b, 384:402, :])

                # t_proj = T @ wt  -> row [1, 512]
                ps_tp = psml_ps.tile([1, 512], F32, tag="pssml")
                for kc in range(4):
                    p = min(KC[kc], 128 if kc < 3 else 16)
                    nc.tensor.matmul(
                        ps_tp[:],
                        lhsT=_r(wcols[0:p, 4 + kc : 5 + kc]),
                        rhs=_r(ttt[0:p, kc, :]),
                        start=(kc == 0),
                        stop=(kc == 3),
                    )
                tp_row = psml.tile([1, 512], F32R, tag="tp_row")
                nc.scalar.copy(tp_row[:], ps_tp[:])

                # s_proj = S @ ws  -> row [1, 512]
                ps_sp = psml_ps.tile([1, 512], F32, tag="pssml")
                for kc in range(4):
                    p = min(KC[kc], 128 if kc < 3 else 16)
                    nc.tensor.matmul(
                        ps_sp[:],
                        lhsT=_r(wcols[0:p, kc : kc + 1]),
                        rhs=_r(stt[0:p, kc, :]),
                        start=(kc == 0),
                        stop=(kc == 3),
                    )
                sp_row = psml.tile([1, 512], F32R, tag="sp_row")
                nc.scalar.copy(sp_row[:], ps_sp[:])

                # scale T^T by wm in place (after t_proj consumed raw T^T)
                for kc in range(4):
                    p = min(KC[kc], 128 if kc < 3 else 16)
                    nc.vector.tensor_scalar_mul(
                        ttt[0:p, kc, :],
                        ttt[0:p, kc, :],
                        wcols[0:p, 8 + kc : 9 + kc].bitcast(F32),
                    )
                # drop the projection rows into the affine contraction rows:
                # tt chunk3 row 400 (partition 16) = t_proj; st row 401 (p17) = s_proj
                # (host already provides the matching ones rows)
                nc.sync.dma_start(out=ttt[16:17, 3, :], in_=tp_row[:])
                nc.sync.dma_start(out=stt[17:18, 3, :], in_=sp_row[:])
                state[b] = (srow, trow, stt, ttt, tp_row, sp_row)

            def simT_pass(b):
                """E^T = exp(sim^T) with t_proj folded in as a rank-1 update."""
                srow, trow, stt, ttt, tp_row, sp_row = state[b]
                et = pet.tile([128, 4, 512], F32R, tag="et")
                for jc in range(4):
                    ps = pbig_ps.tile([128, 512], F32, tag="psbig")
                    for kc in range(4):
                        p = KC[kc]
                        nc.tensor.matmul(
                            ps[:],
                            lhsT=_r(ttt[0:p, kc, ts(jc, 128)]),
                            rhs=_r(stt[0:p, kc, :]),
                            start=(kc == 0),
                            stop=(kc == 3),
                        )
                    nc.scalar.activation(et[:, jc, :], ps[:], EXP)
                state[b] = state[b] + (et,)

            def rest(b):
                srow, trow, stt, ttt, tp_row, sp_row, et = state[b]
                # sim pass: full row max over j (t_proj and s_proj as rank-1s)
                mtile = ptiny.tile([128, 4], F32, tag="mtile")
                for ic in range(4):
                    ps = pbig_ps.tile([128, 512], F32, tag="psbig")
                    for kc in range(4):
                        p = KC[kc]
                        nc.tensor.matmul(
                            ps[:],
                            lhsT=_r(stt[0:p, kc, ts(ic, 128)]),
                            rhs=_r(ttt[0:p, kc, :]),
                            start=(kc == 0),
                            stop=(kc == 3),
                        )
                    nc.vector.reduce_max(mtile[:, ic : ic + 1], ps[:], axis=AX)

                # w = exp(row max)
                wtile = ptiny.tile([128, 4], F32R, tag="wtile")
                nc.scalar.activation(wtile[:], mtile[:], EXP)

                # target_source = (w @ [S|1]) / sum(w)   (plain fp32 matmuls)
                ps_ts = psml_ps.tile([1, 402], F32, tag="pssml")
                for ic in range(4):
                    nc.tensor.matmul(
                        ps_ts[:],
                        lhsT=wtile[:, ic : ic + 1],
                        rhs=srow[:, ic, 0:402],
                        start=(ic == 0),
                        stop=(ic == 3),
                    )
                rts = ptiny.tile([1, 1], F32, tag="rts")
                nc.vector.reciprocal(rts[:], ps_ts[0:1, 400:401])
                tsn = psml.tile([1, 400], F32R, tag="tsn")
                nc.scalar.mul(tsn[:], ps_ts[0:1, 0:400], rts[:])
                ps_tsb = psml_ps.tile([128, 400], F32, tag="pssml")
                nc.tensor.matmul(
                    ps_tsb[:],
                    lhsT=_r(ones_row[0:1, 0:128]),
                    rhs=tsn[:],
                    start=True,
                    stop=True,
                )

                # epilogue per i-chunk: A @ [T|1] with deferred softmax scale
                stf = pout.tile([128, 4, 400], F32, tag="stf")
                sxst = pout.tile([128, 4, 400], F32, tag="sxst")
                sxts = pout.tile([128, 4, 400], F32, tag="sxts")
                for ic in range(4):
                    po = pbig_ps.tile([128, 512], F32, tag="psbig")
                    for jc in range(4):
                        nc.tensor.matmul(
                            po[:, 0:402],
                            lhsT=_r(et[:, jc, ts(ic, 128)]),
                            rhs=_r(trow[:, jc, 0:402]),
                            start=(jc == 0),
                            stop=(jc == 3),
                        )
                    rinv = ptiny.tile([128, 1], F32, tag="rinv")
                    nc.vector.reciprocal(rinv[:], po[:, 400:401])
                    # source_target = (E^T.T @ T) / rowsum
                    nc.scalar.mul(stf[:, ic, :], po[:, 0:400], rinv[:])
                    # S * target_source (tsb broadcast in PSUM)
                    nc.vector.tensor_mul(
                        sxts[:, ic, :], srow[:, ic, 0:400].bitcast(F32), ps_tsb[:]
                    )
                    # S * source_target (on gpsimd: SBUF x SBUF)
                    nc.gpsimd.tensor_mul(
                        sxst[:, ic, :], srow[:, ic, 0:400].bitcast(F32), stf[:, ic, :]
                    )

                # merged output DMAs: one per 400-wide piece
                pieces = (srow[:, :, 0:400].bitcast(F32), stf[:], sxst[:], sxts[:])
                for q, piece in enumerate(pieces):
                    nc.scalar.dma_start(
                        out=out_h[b, :, 400 * q : 400 * (q + 1)].rearrange(
                            "(i p) c -> p i c", p=128
                        ),
                        in_=piece,
                    )

            prologue(0)
            for b in range(BL):
                simT_pass(b)
                if b + 1 < BL:
                    prologue(b + 1)
                rest(b)
    return nc


_NC_CACHE: list = []


def _get_program() -> bass.Bass:
    if not _NC_CACHE:
        nc = build_program()
        _split_multi_waits(nc)
        _NC_CACHE.append(nc)
    return _NC_CACHE[0]


def _host_shards(S: np.ndarray, T: np.ndarray, w: np.ndarray):
    """Build per-core input maps (pure layout marshalling, no math)."""
    ws, wt, wm = w[:D], w[D : 2 * D], w[2 * D :]
    wcols = np.zeros((128, 12), np.float32)
    for kc in range(4):
        p = 128 if kc < 3 else 16
        wcols[0:p, kc] = ws[kc * 128 : kc * 128 + p]
        wcols[0:p, 4 + kc] = wt[kc * 128 : kc * 128 + p]
        wcols[0:p, 8 + kc] = wm[kc * 128 : kc * 128 + p]

    def aug_rows(X):  # [bl, L, 400] -> [bl, L, 404] with col 400 = 1.0
        bl = X.shape[0]
        out = np.zeros((bl, X.shape[1], 404), np.float32)
        out[:, :, 0:400] = X
        out[:, :, 400] = 1.0
        return out

    def aug_t(X, ones_at):  # [bl, L, 400] -> [bl, 402, L] transposed + affine rows
        bl, L, _ = X.shape
        out = np.zeros((bl, 402, L), np.float32)
        out[:, 0:400, :] = X.transpose(0, 2, 1)
        out[:, ones_at, :] = 1.0
        return out

    in_maps = []
    for c in range(N_CORES):
        Sb = np.ascontiguousarray(S[c * BL : (c + 1) * BL])
        Tb = np.ascontiguousarray(T[c * BL : (c + 1) * BL])
        in_maps.append(
            {
                "srow": aug_rows(Sb),
                "trow": aug_rows(Tb),
                "st": aug_t(Sb, ones_at=400),
                "tt": aug_t(Tb, ones_at=401),
                "wcols": wcols,
                "onesr": np.ones((1, 512), np.float32),
            }
        )
    return in_maps


def kernel(source_embedding, target_embedding, w_sim, **run_kwargs):
    S = np.asarray(source_embedding, dtype=np.float32)
    T = np.asarray(target_embedding, dtype=np.float32)
    w = np.asarray(w_sim, dtype=np.float32)
    assert S.shape == (B, LS, D) and T.shape == (B, LT, D) and w.shape == (3 * D,)

    nc = _get_program()
    in_maps = _host_shards(S, T, w)
    res = run_bass_kernel_spmd(nc, in_maps, core_ids=list(range(N_CORES)), **run_kwargs)
    out = np.concatenate([res.results[c]["out"] for c in range(N_CORES)], axis=0)
    if run_kwargs:
        kernel.last_results = res  # expose profile info to test harness
    return out


# revision 16
# speedup vs baseline: 1.1940x; 1.1940x over previous
"""BiDAF attention-flow kernel for 8 Trainium2 NeuronCores (Bass/Tile).

Data-parallel over batch: B=32 -> 4 batches per core on 8 cores.

Math (per batch b):
  sim[i,j] = s_proj[i] + t_proj[j] + sum_d S[i,d]*wm[d]*T[j,d]
  A        = softmax_j(sim)            (row-constant s_proj cancels)
  source_target = A @ T
  w[i]     = exp(max_j sim[i,j]) ; tgt_attn = w / sum(w)
  target_source = tgt_attn @ S         (one row, broadcast over rows)
  out      = [S | source_target | S*source_target | S*target_source]

Device strategy:
  - Host pre-transposes S,T to d-major layout (pure layout marshalling) so the
    similarity matmuls (contraction over d) need no on-device transposes.
  - sim^T (j on partitions) computed on PE -> exp on ACT with per-partition
    bias = t_proj[j]  -> E^T used as lhsT of the A@T matmul.
  - sim (i on partitions) computed on PE (plus rank-1 t_proj broadcast via a
    k=1 matmul) only to get the row max on DVE.
  - softmax normalization deferred: a ones-column appended to T gives the row
    sums in column 400 of the A@T matmul output; scaled on ACT via
    activation(Copy, scale=1/rowsum).
  - All big matmuls run as float32r (full PE rate for fp32 data).
"""

import sys
import types

import numpy as np
import ml_dtypes

# concourse is importable via the axon sitecustomize path; fall back to /opt.
try:
    import concourse.bass as bass
except ImportError:  # pragma: no cover
    sys.path.insert(0, "/opt/trn_rl_repo")
    import concourse.bass as bass

import concourse.mybir as mybir
import concourse.tile as tile
from concourse.bass import ts
from concourse.bass_utils import run_bass_kernel_spmd

B, LS, LT, D = 32, 512, 512, 400
N_CORES = 8
BL = B // N_CORES  # batches per core
F32 = mybir.dt.float32
F32R = mybir.dt.float32r
BF16 = mybir.dt.bfloat16
EXP = mybir.ActivationFunctionType.Exp
AX = mybir.AxisListType.X


def _split_multi_waits(nc: bass.Bass) -> None:
    """This walrus build encodes at most ONE sync-wait per instruction
    ("Too many sync wait commands" in setupSyncWait).  Tile's wait pass can
    attach several sem-waits to one instruction; hoist the extras onto
    same-engine NoOp carriers immediately before it (the NX sequencer
    executes the waits in order, so semantics are identical)."""
    ctr = 0
    for fn in nc.m.functions:
        for bb in fn.blocks:
            if not any(
                i.sync_info is not None and len(i.sync_info.on_wait) > 1
                for i in bb.instructions
            ):
                continue
            new_insts = []
            for inst in bb.instructions:
                si = inst.sync_info
                if si is not None and len(si.on_wait) > 1:
                    waits = list(si.on_wait)
                    for w in waits[:-1]:
                        ctr += 1
                        nop = mybir.InstDrain(
                            name=f"splitw-{ctr}",
                            engine=inst.engine,
                            sync_info=mybir.SyncInfo(on_wait=[w], on_update=[]),
                            bass_is_fusable=False,
                        )
                        nc.register_instruction(nop, overwrite=True)
                        new_insts.append(nop)
                    del si.on_wait[:-1]
                new_insts.append(inst)
            bb.instructions[:] = new_insts


def _r(ap):
    """View an fp32 AP as float32r so the matmul runs at full PE rate."""
    if ap.dtype == F32R:
        return ap
    return ap.bitcast(F32R)


def build_program() -> bass.Bass:
    nc = bass.Bass("TRN2", target_bir_lowering=False, debug=False)

    # Per-core DRAM I/O (host feeds per-core shards).
    srow_h = nc.dram_tensor("srow", [BL, LS, 404], F32R, kind="ExternalInput").ap()
    trow_h = nc.dram_tensor("trow", [BL, LT, 404], BF16, kind="ExternalInput").ap()
    st_h = nc.dram_tensor("st", [BL, D + 2, LS], BF16, kind="ExternalInput").ap()
    tt_h = nc.dram_tensor("tt", [BL, D + 2, LT], BF16, kind="ExternalInput").ap()
    wcols_h = nc.dram_tensor("wcols", [128, 8], BF16, kind="ExternalInput").ap()
    wmf_h = nc.dram_tensor("wmf", [128, 4], F32, kind="ExternalInput").ap()
    ones_h = nc.dram_tensor("onesr", [1, 512], F32R, kind="ExternalInput").ap()
    out_h = nc.dram_tensor("out", [BL, LS, 1600], F32, kind="ExternalOutput").ap()

    KC = [128, 128, 128, 18]  # d chunks + 2 affine rows (402 = 3*128 + 18)

    with tile.TileContext(nc) as tc:
        with (
            tc.tile_pool(name="singles", bufs=1) as singles,
            tc.tile_pool(name="pin", bufs=4) as pin,
            tc.tile_pool(name="pet", bufs=4) as pet,
            tc.tile_pool(name="pout", bufs=3) as pout,
            tc.tile_pool(name="psml", bufs=3) as psml,
            tc.tile_pool(name="ptiny", bufs=6) as ptiny,
            tc.tile_pool(name="pbig_ps", bufs=4, space="PSUM") as pbig_ps,
            tc.tile_pool(name="psml_ps", bufs=3, space="PSUM") as psml_ps,
        ):
            ones_row = singles.tile([1, 512], F32R)
            nc.sync.dma_start(out=ones_row[:], in_=ones_h)
            wcols = singles.tile([128, 8], BF16)
            nc.sync.dma_start(out=wcols[:], in_=wcols_h)
            wmf = singles.tile([128, 4], F32)
            nc.sync.dma_start(out=wmf[:], in_=wmf_h)

            state = {}

            def prologue(b):
                """Batch b inputs + projections; emitted one batch ahead so
                ACT/DVE/DMA prep overlaps the previous batch's PE main work."""
                srow = pin.tile([128, 4, 404], F32R, tag="srow")
                nc.sync.dma_start(
                    out=srow[:], in_=srow_h[b].rearrange("(i p) c -> p i c", p=128)
                )
                trow = pin.tile([128, 4, 404], BF16, tag="trow")
                nc.sync.dma_start(
                    out=trow[:], in_=trow_h[b].rearrange("(j p) c -> p j c", p=128)
                )
                stt = pin.tile([128, 4, 512], BF16, tag="stt")
                ttt = pin.tile([128, 4, 512], BF16, tag="ttt")
                if b == 0:
                    for kc in range(3):
                        nc.sync.dma_start(
                            out=ttt[:, kc, :], in_=tt_h[b, ts(kc, 128), :]
                        )
                    for kc in range(3):
                        nc.sync.dma_start(
                            out=stt[:, kc, :], in_=st_h[b, ts(kc, 128), :]
                        )
                else:
                    nc.sync.dma_start(
                        out=stt[:, 0:3, :],
                        in_=st_h[b, 0:384, :].rearrange("(k p) c -> p k c", p=128),
                    )
                    nc.sync.dma_start(
                        out=ttt[:, 0:3, :],
                        in_=tt_h[b, 0:384, :].rearrange("(k p) c -> p k c", p=128),
                    )
                nc.sync.dma_start(out=stt[0:18, 3, :], in_=st_h[b, 384:402, :])
                nc.sync.dma_start(out=ttt[0:18, 3, :], in_=tt_h[b, 384:402, :])

                # t_proj = T @ wt  -> row [1, 512]
                ps_tp = psml_ps.tile([1, 512], F32, tag="pssml")
                for kc in range(4):
                    p = min(KC[kc], 128 if kc < 3 else 16)
                    nc.tensor.matmul(
                        ps_tp[:],
                        lhsT=wcols[0:p, 4 + kc : 5 + kc],
                        rhs=ttt[0:p, kc, :],
                        start=(kc == 0),
                        stop=(kc == 3),
                    )
                tp_row = psml.tile([1, 512], BF16, tag="tp_row")
                nc.scalar.copy(tp_row[:], ps_tp[:])

                # s_proj = S @ ws  -> row [1, 512]
                ps_sp = psml_ps.tile([1, 512], F32, tag="pssml")
                for kc in range(4):
                    p = min(KC[kc], 128 if kc < 3 else 16)
                    nc.tensor.matmul(
                        ps_sp[:],
                        lhsT=wcols[0:p, kc : kc + 1],
                        rhs=stt[0:p, kc, :],
                        start=(kc == 0),
                        stop=(kc == 3),
                    )
                sp_row = psml.tile([1, 512], BF16, tag="sp_row")
                nc.scalar.copy(sp_row[:], ps_sp[:])

                # scale T^T by wm in place (after t_proj consumed raw T^T)
                for kc in range(4):
                    p = min(KC[kc], 128 if kc < 3 else 16)
                    nc.vector.tensor_scalar_mul(
                        ttt[0:p, kc, :],
                        ttt[0:p, kc, :],
                        wmf[0:p, kc : kc + 1],
                    )
                # drop the projection rows into the affine contraction rows:
                # tt chunk3 row 400 (partition 16) = t_proj; st row 401 (p17) = s_proj
                # (host already provides the matching ones rows)
                nc.sync.dma_start(out=ttt[16:17, 3, :], in_=tp_row[:])
                nc.sync.dma_start(out=stt[17:18, 3, :], in_=sp_row[:])
                state[b] = (srow, trow, stt, ttt, tp_row, sp_row)

            def simT_pass(b):
                """E^T = exp(sim^T) with t_proj folded in as a rank-1 update."""
                srow, trow, stt, ttt, tp_row, sp_row = state[b]
                et = pet.tile([128, 4, 512], BF16, tag="et")
                for jc in range(4):
                    ps = pbig_ps.tile([128, 512], F32, tag="psbig")
                    for kc in range(4):
                        p = KC[kc]
                        nc.tensor.matmul(
                            ps[:],
                            lhsT=ttt[0:p, kc, ts(jc, 128)],
                            rhs=stt[0:p, kc, :],
                            start=(kc == 0),
                            stop=(kc == 3),
                        )
                    nc.scalar.activation(et[:, jc, :], ps[:], EXP)
                state[b] = state[b] + (et,)

            def rest(b):
                srow, trow, stt, ttt, tp_row, sp_row, et = state[b]
                # sim pass: full row max over j (t_proj and s_proj as rank-1s)
                mtile = ptiny.tile([128, 4], F32, tag="mtile")
                for ic in range(4):
                    ps = pbig_ps.tile([128, 512], F32, tag="psbig")
                    for kc in range(4):
                        p = KC[kc]
                        nc.tensor.matmul(
                            ps[:],
                            lhsT=stt[0:p, kc, ts(ic, 128)],
                            rhs=ttt[0:p, kc, :],
                            start=(kc == 0),
                            stop=(kc == 3),
                        )
                    nc.vector.reduce_max(mtile[:, ic : ic + 1], ps[:], axis=AX)

                # w = exp(row max)
                wtile = ptiny.tile([128, 4], F32R, tag="wtile")
                nc.scalar.activation(wtile[:], mtile[:], EXP)

                # target_source = (w @ [S|1]) / sum(w)   (plain fp32 matmuls)
                ps_ts = psml_ps.tile([1, 402], F32, tag="pssml")
                for ic in range(4):
                    nc.tensor.matmul(
                        ps_ts[:],
                        lhsT=wtile[:, ic : ic + 1],
                        rhs=srow[:, ic, 0:402],
                        start=(ic == 0),
                        stop=(ic == 3),
                    )
                rts = ptiny.tile([1, 1], F32, tag="rts")
                nc.vector.reciprocal(rts[:], ps_ts[0:1, 400:401])
                tsn = psml.tile([1, 400], F32R, tag="tsn")
                nc.scalar.mul(tsn[:], ps_ts[0:1, 0:400], rts[:])
                ps_tsb = psml_ps.tile([128, 400], F32, tag="pssml")
                nc.tensor.matmul(
                    ps_tsb[:],
                    lhsT=_r(ones_row[0:1, 0:128]),
                    rhs=tsn[:],
                    start=True,
                    stop=True,
                )

                # epilogue per i-chunk: A @ [T|1] with deferred softmax scale
                stf = pout.tile([128, 4, 400], F32, tag="stf")
                sxst = pout.tile([128, 4, 400], F32, tag="sxst")
                sxts = pout.tile([128, 4, 400], F32, tag="sxts")
                for ic in range(4):
                    po = pbig_ps.tile([128, 512], F32, tag="psbig")
                    for jc in range(4):
                        nc.tensor.matmul(
                            po[:, 0:402],
                            lhsT=et[:, jc, ts(ic, 128)],
                            rhs=trow[:, jc, 0:402],
                            start=(jc == 0),
                            stop=(jc == 3),
                        )
                    rinv = ptiny.tile([128, 1], F32, tag="rinv")
                    nc.vector.reciprocal(rinv[:], po[:, 400:401])
                    # source_target = (E^T.T @ T) / rowsum
                    nc.scalar.mul(stf[:, ic, :], po[:, 0:400], rinv[:])
                    # S * target_source (tsb broadcast in PSUM)
                    nc.vector.tensor_mul(
                        sxts[:, ic, :], srow[:, ic, 0:400].bitcast(F32), ps_tsb[:]
                    )
                    # S * source_target (on gpsimd: SBUF x SBUF)
                    nc.gpsimd.tensor_mul(
                        sxst[:, ic, :], srow[:, ic, 0:400].bitcast(F32), stf[:, ic, :]
                    )

                # merged output DMAs: one per 400-wide piece
                pieces = (srow[:, :, 0:400].bitcast(F32), stf[:], sxst[:], sxts[:])
                for q, piece in enumerate(pieces):
                    nc.scalar.dma_start(
                        out=out_h[b, :, 400 * q : 400 * (q + 1)].rearrange(
                            "(i p) c -> p i c", p=128
                        ),
                        in_=piece,
                    )

            prologue(0)
            for b in range(BL):
                simT_pass(b)
                if b + 1 < BL:
                    prologue(b + 1)
                rest(b)
    return nc


_NC_CACHE: list = []


def _get_program() -> bass.Bass:
    if not _NC_CACHE:
        nc = build_program()
        _split_multi_waits(nc)
        _NC_CACHE.append(nc)
    return _NC_CACHE[0]


def _host_shards(S: np.ndarray, T: np.ndarray, w: np.ndarray):
    """Build per-core input maps (pure layout marshalling, no math)."""
    ws, wt, wm = w[:D], w[D : 2 * D], w[2 * D :]
    wcols = np.zeros((128, 8), np.float32)
    wmf = np.zeros((128, 4), np.float32)
    for kc in range(4):
        p = 128 if kc < 3 else 16
        wcols[0:p, kc] = ws[kc * 128 : kc * 128 + p]
        wcols[0:p, 4 + kc] = wt[kc * 128 : kc * 128 + p]
        wmf[0:p, kc] = wm[kc * 128 : kc * 128 + p]
    wcols = wcols.astype(ml_dtypes.bfloat16)

    def aug_rows(X):  # [bl, L, 400] -> [bl, L, 404] with col 400 = 1.0
        bl = X.shape[0]
        out = np.zeros((bl, X.shape[1], 404), np.float32)
        out[:, :, 0:400] = X
        out[:, :, 400] = 1.0
        return out

    def aug_t(X, ones_at):  # [bl, L, 400] -> [bl, 402, L] transposed + affine rows
        bl, L, _ = X.shape
        out = np.zeros((bl, 402, L), np.float32)
        out[:, 0:400, :] = X.transpose(0, 2, 1)
        out[:, ones_at, :] = 1.0
        return out.astype(ml_dtypes.bfloat16)

    in_maps = []
    for c in range(N_CORES):
        Sb = np.ascontiguousarray(S[c * BL : (c + 1) * BL])
        Tb = np.ascontiguousarray(T[c * BL : (c + 1) * BL])
        in_maps.append(
            {
                "srow": aug_rows(Sb),
                "trow": aug_rows(Tb).astype(ml_dtypes.bfloat16),
                "st": aug_t(Sb, ones_at=400),
                "tt": aug_t(Tb, ones_at=401),
                "wcols": wcols,
                "wmf": wmf,
                "onesr": np.ones((1, 512), np.float32),
            }
        )
    return in_maps


def kernel(source_embedding, target_embedding, w_sim, **run_kwargs):
    S = np.asarray(source_embedding, dtype=np.float32)
    T = np.asarray(target_embedding, dtype=np.float32)
    w = np.asarray(w_sim, dtype=np.float32)
    assert S.shape == (B, LS, D) and T.shape == (B, LT, D) and w.shape == (3 * D,)

    nc = _get_program()
    in_maps = _host_shards(S, T, w)
    res = run_bass_kernel_spmd(nc, in_maps, core_ids=list(range(N_CORES)), **run_kwargs)
    out = np.concatenate([res.results[c]["out"] for c in range(N_CORES)], axis=0)
    if run_kwargs:
        kernel.last_results = res  # expose profile info to test harness
    return out


# revision 17
# speedup vs baseline: 1.2349x; 1.0342x over previous
"""BiDAF attention-flow kernel for 8 Trainium2 NeuronCores (Bass/Tile).

Data-parallel over batch: B=32 -> 4 batches per core on 8 cores.

Math (per batch b):
  sim[i,j] = s_proj[i] + t_proj[j] + sum_d S[i,d]*wm[d]*T[j,d]
  A        = softmax_j(sim)            (row-constant s_proj cancels)
  source_target = A @ T
  w[i]     = exp(max_j sim[i,j]) ; tgt_attn = w / sum(w)
  target_source = tgt_attn @ S         (one row, broadcast over rows)
  out      = [S | source_target | S*source_target | S*target_source]

Device strategy:
  - Host pre-transposes S,T to d-major layout (pure layout marshalling) so the
    similarity matmuls (contraction over d) need no on-device transposes.
  - sim^T (j on partitions) computed on PE -> exp on ACT with per-partition
    bias = t_proj[j]  -> E^T used as lhsT of the A@T matmul.
  - sim (i on partitions) computed on PE (plus rank-1 t_proj broadcast via a
    k=1 matmul) only to get the row max on DVE.
  - softmax normalization deferred: a ones-column appended to T gives the row
    sums in column 400 of the A@T matmul output; scaled on ACT via
    activation(Copy, scale=1/rowsum).
  - All big matmuls run as float32r (full PE rate for fp32 data).
"""

import sys
import types

import numpy as np
import ml_dtypes

# concourse is importable via the axon sitecustomize path; fall back to /opt.
try:
    import concourse.bass as bass
except ImportError:  # pragma: no cover
    sys.path.insert(0, "/opt/trn_rl_repo")
    import concourse.bass as bass

import concourse.mybir as mybir
import concourse.tile as tile
from concourse.bass import ts
from concourse.bass_utils import run_bass_kernel_spmd

B, LS, LT, D = 32, 512, 512, 400
N_CORES = 8
BL = B // N_CORES  # batches per core
F32 = mybir.dt.float32
F32R = mybir.dt.float32r
BF16 = mybir.dt.bfloat16
EXP = mybir.ActivationFunctionType.Exp
AX = mybir.AxisListType.X


def _split_multi_waits(nc: bass.Bass) -> None:
    """This walrus build encodes at most ONE sync-wait per instruction
    ("Too many sync wait commands" in setupSyncWait).  Tile's wait pass can
    attach several sem-waits to one instruction; hoist the extras onto
    same-engine NoOp carriers immediately before it (the NX sequencer
    executes the waits in order, so semantics are identical)."""
    ctr = 0
    for fn in nc.m.functions:
        for bb in fn.blocks:
            if not any(
                i.sync_info is not None and len(i.sync_info.on_wait) > 1
                for i in bb.instructions
            ):
                continue
            new_insts = []
            for inst in bb.instructions:
                si = inst.sync_info
                if si is not None and len(si.on_wait) > 1:
                    waits = list(si.on_wait)
                    for w in waits[:-1]:
                        ctr += 1
                        nop = mybir.InstNoOp(
                            name=f"splitw-{ctr}",
                            engine=inst.engine,
                            sync_info=mybir.SyncInfo(on_wait=[w], on_update=[]),
                            bass_nofuse=True,
                        )
                        nc.register_instruction(nop, overwrite=True)
                        new_insts.append(nop)
                    del si.on_wait[:-1]
                new_insts.append(inst)
            bb.instructions[:] = new_insts


def _r(ap):
    """View an fp32 AP as float32r so the matmul runs at full PE rate."""
    if ap.dtype == F32R:
        return ap
    return ap.bitcast(F32R)


def build_program() -> bass.Bass:
    nc = bass.Bass("TRN2", target_bir_lowering=False, debug=False)

    # Per-core DRAM I/O (host feeds per-core shards).
    srow_h = nc.dram_tensor("srow", [BL, LS, 404], F32R, kind="ExternalInput").ap()
    trow_h = nc.dram_tensor("trow", [BL, LT, 404], BF16, kind="ExternalInput").ap()
    st_h = nc.dram_tensor("st", [BL, D + 2, LS], BF16, kind="ExternalInput").ap()
    tt_h = nc.dram_tensor("tt", [BL, D + 2, LT], BF16, kind="ExternalInput").ap()
    wcols_h = nc.dram_tensor("wcols", [128, 8], BF16, kind="ExternalInput").ap()
    wmf_h = nc.dram_tensor("wmf", [128, 4], F32, kind="ExternalInput").ap()
    ones_h = nc.dram_tensor("onesr", [1, 512], F32R, kind="ExternalInput").ap()
    out_h = nc.dram_tensor("out", [BL, LS, 1600], F32, kind="ExternalOutput").ap()

    KC = [128, 128, 128, 18]  # d chunks + 2 affine rows (402 = 3*128 + 18)

    with tile.TileContext(nc) as tc:
        with (
            tc.tile_pool(name="singles", bufs=1) as singles,
            tc.tile_pool(name="pin", bufs=4) as pin,
            tc.tile_pool(name="pet", bufs=4) as pet,
            tc.tile_pool(name="pout", bufs=3) as pout,
            tc.tile_pool(name="psml", bufs=3) as psml,
            tc.tile_pool(name="ptiny", bufs=6) as ptiny,
            tc.tile_pool(name="pbig_ps", bufs=4, space="PSUM") as pbig_ps,
            tc.tile_pool(name="psml_ps", bufs=3, space="PSUM") as psml_ps,
        ):
            ones_row = singles.tile([1, 512], F32R)
            nc.sync.dma_start(out=ones_row[:], in_=ones_h)
            wcols = singles.tile([128, 8], BF16)
            nc.sync.dma_start(out=wcols[:], in_=wcols_h)
            wmf = singles.tile([128, 4], F32)
            nc.sync.dma_start(out=wmf[:], in_=wmf_h)

            state = {}

            def prologue(b):
                """Batch b inputs + projections; emitted one batch ahead so
                ACT/DVE/DMA prep overlaps the previous batch's PE main work."""
                srow = pin.tile([128, 4, 404], F32R, tag="srow")
                nc.sync.dma_start(
                    out=srow[:], in_=srow_h[b].rearrange("(i p) c -> p i c", p=128)
                )
                trow = pin.tile([128, 4, 404], BF16, tag="trow")
                nc.sync.dma_start(
                    out=trow[:], in_=trow_h[b].rearrange("(j p) c -> p j c", p=128)
                )
                stt = pin.tile([128, 4, 512], BF16, tag="stt")
                ttt = pin.tile([128, 4, 512], BF16, tag="ttt")
                if b == 0:
                    for kc in range(3):
                        nc.sync.dma_start(
                            out=ttt[:, kc, :], in_=tt_h[b, ts(kc, 128), :]
                        )
                    for kc in range(3):
                        nc.scalar.dma_start(
                            out=stt[:, kc, :], in_=st_h[b, ts(kc, 128), :]
                        )
                else:
                    nc.sync.dma_start(
                        out=stt[:, 0:3, :],
                        in_=st_h[b, 0:384, :].rearrange("(k p) c -> p k c", p=128),
                    )
                    nc.sync.dma_start(
                        out=ttt[:, 0:3, :],
                        in_=tt_h[b, 0:384, :].rearrange("(k p) c -> p k c", p=128),
                    )
                nc.sync.dma_start(out=stt[0:18, 3, :], in_=st_h[b, 384:402, :])
                nc.sync.dma_start(out=ttt[0:18, 3, :], in_=tt_h[b, 384:402, :])

                # t_proj = T @ wt  -> row [1, 512]
                ps_tp = psml_ps.tile([1, 512], F32, tag="pssml")
                for kc in range(4):
                    p = min(KC[kc], 128 if kc < 3 else 16)
                    nc.tensor.matmul(
                        ps_tp[:],
                        lhsT=wcols[0:p, 4 + kc : 5 + kc],
                        rhs=ttt[0:p, kc, :],
                        start=(kc == 0),
                        stop=(kc == 3),
                    )
                tp_row = psml.tile([1, 512], BF16, tag="tp_row")
                nc.scalar.copy(tp_row[:], ps_tp[:])

                # s_proj = S @ ws  -> row [1, 512]
                ps_sp = psml_ps.tile([1, 512], F32, tag="pssml")
                for kc in range(4):
                    p = min(KC[kc], 128 if kc < 3 else 16)
                    nc.tensor.matmul(
                        ps_sp[:],
                        lhsT=wcols[0:p, kc : kc + 1],
                        rhs=stt[0:p, kc, :],
                        start=(kc == 0),
                        stop=(kc == 3),
                    )
                sp_row = psml.tile([1, 512], BF16, tag="sp_row")
                nc.scalar.copy(sp_row[:], ps_sp[:])

                # scale T^T by wm in place (after t_proj consumed raw T^T)
                for kc in range(4):
                    p = min(KC[kc], 128 if kc < 3 else 16)
                    nc.vector.tensor_scalar_mul(
                        ttt[0:p, kc, :],
                        ttt[0:p, kc, :],
                        wmf[0:p, kc : kc + 1],
                    )
                # drop the projection rows into the affine contraction rows:
                # tt chunk3 row 400 (partition 16) = t_proj; st row 401 (p17) = s_proj
                # (host already provides the matching ones rows)
                nc.sync.dma_start(out=ttt[16:17, 3, :], in_=tp_row[:])
                nc.sync.dma_start(out=stt[17:18, 3, :], in_=sp_row[:])
                state[b] = (srow, trow, stt, ttt, tp_row, sp_row)

            def simT_pass(b):
                """E^T = exp(sim^T) with t_proj folded in as a rank-1 update."""
                srow, trow, stt, ttt, tp_row, sp_row = state[b]
                et = pet.tile([128, 4, 512], BF16, tag="et")
                for jc in range(4):
                    ps = pbig_ps.tile([128, 512], F32, tag="psbig")
                    for kc in range(4):
                        p = KC[kc]
                        nc.tensor.matmul(
                            ps[:],
                            lhsT=ttt[0:p, kc, ts(jc, 128)],
                            rhs=stt[0:p, kc, :],
                            start=(kc == 0),
                            stop=(kc == 3),
                        )
                    nc.scalar.activation(et[:, jc, :], ps[:], EXP)
                state[b] = state[b] + (et,)

            def rest(b):
                srow, trow, stt, ttt, tp_row, sp_row, et = state[b]
                # sim pass: full row max over j (t_proj and s_proj as rank-1s)
                mtile = ptiny.tile([128, 4], F32, tag="mtile")
                for ic in range(4):
                    ps = pbig_ps.tile([128, 512], F32, tag="psbig")
                    for kc in range(4):
                        p = KC[kc]
                        nc.tensor.matmul(
                            ps[:],
                            lhsT=stt[0:p, kc, ts(ic, 128)],
                            rhs=ttt[0:p, kc, :],
                            start=(kc == 0),
                            stop=(kc == 3),
                        )
                    nc.vector.reduce_max(mtile[:, ic : ic + 1], ps[:], axis=AX)

                # w = exp(row max)
                wtile = ptiny.tile([128, 4], F32R, tag="wtile")
                nc.scalar.activation(wtile[:], mtile[:], EXP)

                # target_source = (w @ [S|1]) / sum(w)   (plain fp32 matmuls)
                ps_ts = psml_ps.tile([1, 402], F32, tag="pssml")
                for ic in range(4):
                    nc.tensor.matmul(
                        ps_ts[:],
                        lhsT=wtile[:, ic : ic + 1],
                        rhs=srow[:, ic, 0:402],
                        start=(ic == 0),
                        stop=(ic == 3),
                    )
                rts = ptiny.tile([1, 1], F32, tag="rts")
                nc.vector.reciprocal(rts[:], ps_ts[0:1, 400:401])
                tsn = psml.tile([1, 400], F32R, tag="tsn")
                nc.scalar.mul(tsn[:], ps_ts[0:1, 0:400], rts[:])
                ps_tsb = psml_ps.tile([128, 400], F32, tag="pssml")
                nc.tensor.matmul(
                    ps_tsb[:],
                    lhsT=_r(ones_row[0:1, 0:128]),
                    rhs=tsn[:],
                    start=True,
                    stop=True,
                )

                # epilogue per i-chunk: A @ [T|1] with deferred softmax scale
                stf = pout.tile([128, 4, 400], F32, tag="stf")
                sxst = pout.tile([128, 4, 400], F32, tag="sxst")
                sxts = pout.tile([128, 4, 400], F32, tag="sxts")
                for ic in range(4):
                    po = pbig_ps.tile([128, 512], F32, tag="psbig")
                    for jc in range(4):
                        nc.tensor.matmul(
                            po[:, 0:402],
                            lhsT=et[:, jc, ts(ic, 128)],
                            rhs=trow[:, jc, 0:402],
                            start=(jc == 0),
                            stop=(jc == 3),
                        )
                    rinv = ptiny.tile([128, 1], F32, tag="rinv")
                    nc.vector.reciprocal(rinv[:], po[:, 400:401])
                    # source_target = (E^T.T @ T) / rowsum
                    nc.vector.tensor_scalar_mul(stf[:, ic, :], po[:, 0:400], rinv[:])
                    # S * target_source (tsb broadcast in PSUM)
                    nc.vector.tensor_mul(
                        sxts[:, ic, :], srow[:, ic, 0:400].bitcast(F32), ps_tsb[:]
                    )
                    # S * source_target (on gpsimd: SBUF x SBUF)
                    nc.gpsimd.tensor_mul(
                        sxst[:, ic, :], srow[:, ic, 0:400].bitcast(F32), stf[:, ic, :]
                    )

                # merged output DMAs: one per 400-wide piece
                pieces = (srow[:, :, 0:400].bitcast(F32), stf[:], sxst[:], sxts[:])
                if b == BL - 1:
                    for ic in range(4):
                        for q, piece in enumerate(pieces):
                            eng = nc.sync if (ic + q) % 2 == 0 else nc.scalar
                            eng.dma_start(
                                out=out_h[b, ts(ic, 128), 400 * q : 400 * (q + 1)],
                                in_=piece[:, ic, :],
                            )
                else:
                    for q, piece in enumerate(pieces):
                        nc.scalar.dma_start(
                            out=out_h[b, :, 400 * q : 400 * (q + 1)].rearrange(
                                "(i p) c -> p i c", p=128
                            ),
                            in_=piece,
                        )

            prologue(0)
            for b in range(BL):
                simT_pass(b)
                if b + 1 < BL:
                    prologue(b + 1)
                rest(b)
    return nc


_NC_CACHE: list = []


def _get_program() -> bass.Bass:
    if not _NC_CACHE:
        nc = build_program()
        _split_multi_waits(nc)
        _NC_CACHE.append(nc)
    return _NC_CACHE[0]


def _host_shards(S: np.ndarray, T: np.ndarray, w: np.ndarray):
    """Build per-core input maps (pure layout marshalling, no math)."""
    ws, wt, wm = w[:D], w[D : 2 * D], w[2 * D :]
    wcols = np.zeros((128, 8), np.float32)
    wmf = np.zeros((128, 4), np.float32)
    for kc in range(4):
        p = 128 if kc < 3 else 16
        wcols[0:p, kc] = ws[kc * 128 : kc * 128 + p]
        wcols[0:p, 4 + kc] = wt[kc * 128 : kc * 128 + p]
        wmf[0:p, kc] = wm[kc * 128 : kc * 128 + p]
    wcols = wcols.astype(ml_dtypes.bfloat16)

    def aug_rows(X):  # [bl, L, 400] -> [bl, L, 404] with col 400 = 1.0
        bl = X.shape[0]
        out = np.zeros((bl, X.shape[1], 404), np.float32)
        out[:, :, 0:400] = X
        out[:, :, 400] = 1.0
        return out

    def aug_t(X, ones_at):  # [bl, L, 400] -> [bl, 402, L] transposed + affine rows
        bl, L, _ = X.shape
        out = np.zeros((bl, 402, L), np.float32)
        out[:, 0:400, :] = X.transpose(0, 2, 1)
        out[:, ones_at, :] = 1.0
        return out.astype(ml_dtypes.bfloat16)

    in_maps = []
    for c in range(N_CORES):
        Sb = np.ascontiguousarray(S[c * BL : (c + 1) * BL])
        Tb = np.ascontiguousarray(T[c * BL : (c + 1) * BL])
        in_maps.append(
            {
                "srow": aug_rows(Sb),
                "trow": aug_rows(Tb).astype(ml_dtypes.bfloat16),
                "st": aug_t(Sb, ones_at=400),
                "tt": aug_t(Tb, ones_at=401),
                "wcols": wcols,
                "wmf": wmf,
                "onesr": np.ones((1, 512), np.float32),
            }
        )
    return in_maps


def kernel(source_embedding, target_embedding, w_sim, **run_kwargs):
    S = np.asarray(source_embedding, dtype=np.float32)
    T = np.asarray(target_embedding, dtype=np.float32)
    w = np.asarray(w_sim, dtype=np.float32)
    assert S.shape == (B, LS, D) and T.shape == (B, LT, D) and w.shape == (3 * D,)

    nc = _get_program()
    in_maps = _host_shards(S, T, w)
    res = run_bass_kernel_spmd(nc, in_maps, core_ids=list(range(N_CORES)), **run_kwargs)
    out = np.concatenate([res.results[c]["out"] for c in range(N_CORES)], axis=0)
    if run_kwargs:
        kernel.last_results = res  # expose profile info to test harness
    return out
